# revision 1
# baseline (speedup 1.0000x reference)
"""CenterLoss on 8 Trainium2 NeuronCores.

loss = mean(distmat * onehot(labels)) over the full (B, C) matrix, where
distmat[i, j] = ||x_i||^2 + ||c_j||^2 - 2 x_i.c_j. The mask keeps only
distmat[i, labels[i]], so the loss reduces exactly to

    sum_i ||x_i - centers[labels[i]]||^2 / (B * C)

Sharding: batch-parallel. Each of the 8 cores takes 128 rows of x and
their labels, gathers the 128 labeled center rows from its DRAM copy of
`centers` with one indirect DMA, computes the per-row squared distance,
reduces to a scalar partial sum on-device, and the host combines the 8
partials and applies the 1/(B*C) mean scaling.

The device program is hand-scheduled Bacc (no TileContext): per-engine
instruction streams with explicit semaphores, so the kernel skips the
Tile entry/exit barrier machinery. Labels ride in as exact-integer f32
in a [1, 128] row (one 512B DMA descriptor), get transposed across
partitions with a K=1 matmul on the idle PE, and are converted to int32
offsets for the SWDGE gather.
"""

import numpy as np

import concourse.bacc as bacc
import concourse.bass as bass
import concourse.mybir as mybir
from concourse.bass_utils import run_bass_kernel_spmd

B = 1024
C = 100000
D = 128
NCORES = 8
BS = B // NCORES  # 128 rows per core == one SBUF partition tile

F32 = mybir.dt.float32
I32 = mybir.dt.int32

_NC_CACHE = {}


def _build_nc():
    nc = bacc.Bacc("TRN2")

    x = nc.dram_tensor("x", [BS, D], F32, kind="ExternalInput")
    labels = nc.dram_tensor("labels", [1, BS], F32, kind="ExternalInput")
    centers = nc.dram_tensor("centers", [C, D], F32, kind="ExternalInput")
    out = nc.dram_tensor("out", [1, 1], F32, kind="ExternalOutput")

    with (
        nc.sbuf_tensor("x_t", [BS, D], F32) as x_t,
        nc.sbuf_tensor("lab_row", [1, BS], F32) as lab_row,
        nc.sbuf_tensor("lab_i32", [BS, 1], I32) as lab_i32,
        nc.sbuf_tensor("c_t", [BS, D], F32) as c_t,
        nc.sbuf_tensor("d_t", [BS, D], F32) as d_t,
        nc.sbuf_tensor("rowsum", [BS, 1], F32) as rowsum,
        nc.sbuf_tensor("ones", [BS, 1], F32) as ones,
        nc.sbuf_tensor("one1", [1, 1], F32) as one1,
        nc.sbuf_tensor("res", [1, 1], F32) as res,
        nc.psum_tensor("lab_ps", [BS, 1], F32) as lab_ps,
        nc.psum_tensor("tot_ps", [1, 1], F32) as tot_ps,
        nc.semaphore("d_lab") as d_lab,
        nc.semaphore("d_x") as d_x,
        nc.semaphore("d_g") as d_g,
        nc.semaphore("d_out") as d_out,
        nc.semaphore("s_pe") as s_pe,
        nc.semaphore("s_dve") as s_dve,
    ):
        block_cm = nc.Block(no_gpsimd_drain=True)
        block = block_cm.__enter__()

        @block.sync
        def _(sync):
            sync.dma_start(lab_row.ap(), labels[:, :]).then_inc(d_lab, 16)
            sync.wait_ge(s_dve, 6)
            sync.dma_start(out[:, :], res.ap()).then_inc(d_out, 16)
            sync.wait_ge(d_out, 16)

        @block.scalar
        def _(scalar):
            scalar.dma_start(x_t.ap(), x[:, :]).then_inc(d_x, 16)

        @block.vector
        def _(vector):
            vector.memset(ones.ap(), 1.0)
            vector.memset(one1.ap(), 1.0).then_inc(s_dve, 1)  # s_dve=1
            # labels transpose (PSUM) -> int32 gather offsets
            vector.wait_ge(s_pe, 1)
            vector.tensor_copy(lab_i32.ap(), lab_ps.ap()).then_inc(s_dve, 1)  # =2
            # main chain (same-engine RAW needs sem handoffs on DVE)
            vector.wait_ge(d_g, 16)
            vector.wait_ge(d_x, 16)
            vector.tensor_sub(d_t.ap(), x_t.ap(), c_t.ap()).then_inc(s_dve, 1)  # =3
            vector.wait_ge(s_dve, 3)
            vector.tensor_mul(d_t.ap(), d_t.ap(), d_t.ap()).then_inc(s_dve, 1)  # =4
            vector.wait_ge(s_dve, 4)
            vector.reduce_sum(
                rowsum.ap(), d_t.ap(), axis=mybir.AxisListType.X
            ).then_inc(s_dve, 1)  # =5
            vector.wait_ge(s_pe, 2)
            vector.tensor_copy(res.ap(), tot_ps.ap()).then_inc(s_dve, 1)  # =6

        @block.tensor
        def _(tensor):
            # Transpose labels across partitions: lab_ps[m,0] = lab_row[0,m].
            tensor.wait_ge(d_lab, 16)
            tensor.wait_ge(s_dve, 1)
            tensor.matmul(
                lab_ps.ap(), lhsT=lab_row.ap(), rhs=one1.ap(), start=True, stop=True
            ).then_inc(s_pe, 1)
            # Partition-axis reduction: ones.T @ rowsum -> [1,1].
            tensor.wait_ge(s_dve, 5)
            tensor.matmul(
                tot_ps.ap(), lhsT=ones.ap(), rhs=rowsum.ap(), start=True, stop=True
            ).then_inc(s_pe, 1)

        @block.gpsimd
        def _(gpsimd):
            # Gather centers[labels[p], :] into partition p of c_t.
            gpsimd.wait_ge(s_dve, 2)
            gpsimd.indirect_dma_start(
                out=c_t.ap(),
                out_offset=None,
                in_=centers[:, :],
                in_offset=bass.IndirectOffsetOnAxis(ap=lab_i32.ap()[:, :1], axis=0),
            ).then_inc(d_g, 16)

        # Block.__exit__ emits per-engine drains + a sem-only barrier.
        # Clear sems after it so the NEFF is safely re-executable.
        block_cm.__exit__(None, None, None)
        for sem in (d_lab, d_x, d_g, d_out, s_pe, s_dve):
            nc.sync.sem_clear(sem)

    nc.compile()
    return nc


def _run(x, labels, centers, **spmd_kwargs):
    """Shard, run on 8 cores, combine. Returns (loss, BassKernelResults)."""
    x = np.ascontiguousarray(np.asarray(x, dtype=np.float32))
    centers = np.ascontiguousarray(np.asarray(centers, dtype=np.float32))
    # labels as exact-integer f32 rows (values < 2^24, so f32 is lossless)
    labels_f32 = np.asarray(labels).astype(np.float32).reshape(NCORES, 1, BS)

    if "nc" not in _NC_CACHE:
        _NC_CACHE["nc"] = _build_nc()
    nc = _NC_CACHE["nc"]

    in_maps = [
        {
            "x": x[i * BS : (i + 1) * BS],
            "labels": np.ascontiguousarray(labels_f32[i]),
            "centers": centers,
        }
        for i in range(NCORES)
    ]
    res = run_bass_kernel_spmd(nc, in_maps, core_ids=list(range(NCORES)), **spmd_kwargs)

    total = float(np.sum([r["out"][0, 0] for r in res.results], dtype=np.float64))
    return np.float32(total / (B * C)), res


def kernel(x, labels, centers):
    loss, _ = _run(x, labels, centers)
    return loss



# revision 3
# speedup vs baseline: 1.1914x; 1.1914x over previous
"""CenterLoss on 8 Trainium2 NeuronCores.

loss = mean(distmat * onehot(labels)) over the full (B, C) matrix, where
distmat[i, j] = ||x_i||^2 + ||c_j||^2 - 2 x_i.c_j. The mask keeps only
distmat[i, labels[i]], so the loss reduces exactly to

    sum_i ||x_i - centers[labels[i]]||^2 / (B * C)

Sharding: batch-parallel. Each of the 8 cores takes 128 rows of x and
their labels, gathers the 128 labeled center rows from its DRAM copy of
`centers` with one indirect DMA, computes the per-row squared distance,
and writes the 128 per-row sums back to DRAM. The host adds the 8x128
partials and applies the 1/(B*C) mean scaling.

The device program is raw per-engine Bacc instruction streams emitted
straight into the entry block -- no Block machinery, no exit barrier, no
explicit sem clears (the NEFF epilogue emitted by the BIR compiler
drains every engine's DMA queues, barriers, and resets every kernel
semaphore anyway). Keeping each engine's stream as short as possible
matters because the epilogue per-engine semaphore-reset runs only after
ALL engines finish their streams.

Critical chain: SP issues the labels DMA (int32 row indices straight
from the host, so no transpose / cast) and the x DMA; Pool waits on the
labels and launches the SWDGE gather; DVE computes (x-c)^2 row sums;
SP DMAs the [128,1] row sums out (no completion semaphore -- the
epilogue drain covers it). PE and ACT run no instructions at all, so
they park at the epilogue barrier immediately.
"""

import numpy as np

import concourse.bacc as bacc
import concourse.bass as bass
import concourse.mybir as mybir
from concourse.bass_utils import run_bass_kernel_spmd

B = 1024
C = 100000
D = 128
NCORES = 8
BS = B // NCORES  # 128 rows per core == one SBUF partition tile

F32 = mybir.dt.float32
I32 = mybir.dt.int32

STRIP_CONST_MEMSETS = True

_NC_CACHE = {}


def _strip_const_memsets(nc):
    """Drop the framework's const-AP init memsets (const-float32-0.0 etc.).

    This kernel never uses the const APs, and those memsets are the first
    named instructions in the program, so removing them both trims dead
    code and lets every engine reach its real stream sooner.
    """
    for f in nc.m.functions:
        for blk in f.blocks:
            blk.instructions = [
                i
                for i in blk.instructions
                if not (
                    type(i).__name__ == "InstMemset"
                    and i.outs
                    and "const-" in str(i.outs[0])
                )
            ]


def _build_nc():
    nc = bacc.Bacc("TRN2")

    x = nc.dram_tensor("x", [BS, D], F32, kind="ExternalInput")
    labels = nc.dram_tensor("labels", [BS, 1], I32, kind="ExternalInput")
    centers = nc.dram_tensor("centers", [C, D], F32, kind="ExternalInput")
    out = nc.dram_tensor("out", [BS, 1], F32, kind="ExternalOutput")

    with (
        nc.sbuf_tensor("x_t", [BS, D], F32) as x_t,
        nc.sbuf_tensor("lab_sb", [BS, 1], I32) as lab_sb,
        nc.sbuf_tensor("c_t", [BS, D], F32) as c_t,
        nc.sbuf_tensor("d_t", [BS, D], F32) as d_t,
        nc.sbuf_tensor("rowsum", [BS, 1], F32) as rowsum,
    ):
        d_lab = nc.alloc_semaphore("d_lab")
        d_x = nc.alloc_semaphore("d_x")
        d_g = nc.alloc_semaphore("d_g")
        d_out = nc.alloc_semaphore("d_out")
        s_dve = nc.alloc_semaphore("s_dve")

        # SP: input DMAs, then park until the row sums are ready.
        nc.sync.dma_start(lab_sb.ap(), labels[:, :]).then_inc(d_lab, 16)
        nc.sync.dma_start(x_t.ap(), x[:, :]).then_inc(d_x, 16)
        nc.sync.wait_ge(s_dve, 3)
        # d_out has no waiter; the epilogue's SP drain covers completion.
        nc.sync.dma_start(out[:, :], rowsum.ap()).then_inc(d_out, 16)

        # Pool: gather centers[labels[p], :] into partition p of c_t.
        nc.gpsimd.wait_ge(d_lab, 16)
        nc.gpsimd.indirect_dma_start(
            out=c_t.ap(),
            out_offset=None,
            in_=centers[:, :],
            in_offset=bass.IndirectOffsetOnAxis(ap=lab_sb.ap()[:, :1], axis=0),
        ).then_inc(d_g, 16)

        # DVE: (x - c)^2 row sums. Same-engine RAW handoffs via s_dve.
        nc.vector.wait_ge(d_x, 16)
        nc.vector.wait_ge(d_g, 16)
        nc.vector.tensor_sub(d_t.ap(), x_t.ap(), c_t.ap()).then_inc(s_dve, 1)
        nc.vector.wait_ge(s_dve, 1)
        nc.vector.tensor_mul(d_t.ap(), d_t.ap(), d_t.ap()).then_inc(s_dve, 1)
        nc.vector.wait_ge(s_dve, 2)
        nc.vector.reduce_sum(
            rowsum.ap(), d_t.ap(), axis=mybir.AxisListType.X
        ).then_inc(s_dve, 1)

    if STRIP_CONST_MEMSETS:
        _strip_const_memsets(nc)
    nc.compile()
    return nc


def _run(x, labels, centers, **spmd_kwargs):
    """Shard, run on 8 cores, combine. Returns (loss, BassKernelResults)."""
    x = np.ascontiguousarray(np.asarray(x, dtype=np.float32))
    centers = np.ascontiguousarray(np.asarray(centers, dtype=np.float32))
    labels_i32 = np.asarray(labels).astype(np.int32).reshape(NCORES, BS, 1)

    if "nc" not in _NC_CACHE:
        _NC_CACHE["nc"] = _build_nc()
    nc = _NC_CACHE["nc"]

    in_maps = [
        {
            "x": x[i * BS : (i + 1) * BS],
            "labels": np.ascontiguousarray(labels_i32[i]),
            "centers": centers,
        }
        for i in range(NCORES)
    ]
    res = run_bass_kernel_spmd(nc, in_maps, core_ids=list(range(NCORES)), **spmd_kwargs)

    total = float(
        np.sum([r["out"].astype(np.float64) for r in res.results], dtype=np.float64)
    )
    return np.float32(total / (B * C)), res


def kernel(x, labels, centers):
    loss, _ = _run(x, labels, centers)
    return loss
